# revision 1
# baseline (speedup 1.0000x reference)
"""Trainium2 Bass kernel for nn_Actor (3 grouped conv1d blocks + dense + tanh).

Sharding: column-parallel across 8 cores. Core j owns input channels
{2j, 2j+1}; because every conv is grouped (depthwise x8 filters), that
slice owns contiguous channel blocks through the whole net:
  conv1 out-ch [16j,16j+16), conv2 out-ch [128j,128j+128),
  conv3 out-ch [1024j, 1024j+1024), and rows {l*8192 + ch} of W.
Each core computes partial dense outputs; the host sums them, adds bd
and applies tanh.

On-chip layout notes:
- batch is carried as b = bg*8 + b8 with bg blocks OUTER in the free
  dim ((bg, l, b8) order) so every inter-stage copy has contiguous
  ~KB runs and reads all 128 SBUF partitions (full AXI port width).
- conv2/conv3/dense run in bf16 (f32 psum accumulate).
- dense packs 4 l' per MM (junk off-diagonal psum blocks never read)
  and round-robins quads over 4 PE column tiles.
"""

import ml_dtypes
import numpy as np

import concourse.bass as bass
import concourse.tile as tile
from concourse import bacc
from concourse import mybir
from concourse.bass_utils import run_bass_kernel_spmd

B = 64
L = 128
C = 16
FILTERS = 8
K = 5
N_CORES = 8

L1 = L - K + 1      # 124
L2 = L1 - K + 1     # 120
L3 = L2 - K + 1     # 116

C0 = C // N_CORES           # 2    input cols per core
C1 = C0 * FILTERS           # 16   conv1 out-ch per core
C2 = C1 * FILTERS           # 128  conv2 out-ch per core
C3 = C2 * FILTERS           # 1024 conv3 out-ch per core
G3 = C2 // C1               # 8    conv3 groups of 16 in-ch
BG = 8                      # batch groups (conv1 partition packing)
B8 = B // BG                # 8

F32 = mybir.dt.float32
BF16 = mybir.dt.bfloat16

_CACHE = {}


def _build_nc(reps=1, mode='full'):
    """Build the SPMD Bass program (same on all 8 cores).

    reps>1 wraps the pipeline in a device-side loop (used only for
    timing by wall-clock differencing).
    """
    nc = bacc.Bacc("TRN2", target_bir_lowering=False, debug=False)

    NB1 = L1 * B8            # 992   conv1 free (l1, b8) per bg block
    NB2 = L2 * B             # 7680  conv2 cols (bg, l2, b8)
    NB3 = L3 * B             # 7424  conv3 cols (bg, l3, b8)

    a1 = nc.declare_dram_parameter("a1", [80, NB1], F32, isOutput=False)
    cs = nc.declare_dram_parameter("cs", [128, 138], F32, isOutput=False)
    sb = nc.declare_dram_parameter("sb", [80, 640 + G3 * 128], BF16, isOutput=False)
    wt = nc.declare_dram_parameter("wt", [128, G3 * L3 * 2], BF16, isOutput=False)
    out = nc.declare_dram_parameter("out", [128, 4 * B], F32, isOutput=True)

    with tile.TileContext(nc) as tc:
        with (
            tc.tile_pool(name="consts", bufs=1) as consts,
            tc.tile_pool(name="work", bufs=1) as work,
            tc.tile_pool(name="i3pool", bufs=4) as i3pool,
            tc.tile_pool(name="zpool", bufs=3) as zpool,
            tc.tile_pool(name="psum", bufs=4, space=bass.MemorySpace.PSUM) as psum,
            tc.tile_pool(name="psumd", bufs=1, space=bass.MemorySpace.PSUM) as psumd,
        ):
            # ---- constants ----
            # cs (f32): s1 [80,128] @0:128, b1 @128, b2 @129, b3 @130:138
            # sb (bf16): s2k 5x[16,128] @0:640, s3 8x[80? (k,c)x16,128] @640+
            a1_t = consts.tile([80, NB1], F32)
            cs_t = consts.tile([128, 138], F32)
            sb_t = consts.tile([80, 640 + G3 * 128], BF16)
            nc.sync.dma_start(a1_t[:], a1[:])
            nc.sync.dma_start(cs_t[:], cs[:])
            nc.sync.dma_start(sb_t[:], sb[:])
            WQ = G3 * L3 * 2 // 4  # 464
            wt_ts = []
            for q in range(4):
                wq = consts.tile([128, WQ], BF16, tag=f"wt{q}")
                nc.sync.dma_start(wq[:], wt[:, q * WQ:(q + 1) * WQ])
                wt_ts.append(wq)

            rep_cm = (tc.For_i(0, reps, 1,
                               hint_engines=(mybir.EngineType.PE,
                                             mybir.EngineType.DVE,
                                             mybir.EngineType.Activation,
                                             mybir.EngineType.SP,
                                             mybir.EngineType.Pool))
                      if reps > 1 else None)
            if rep_cm is not None:
                rep_cm.__enter__()
            for _rep in range(1):
              # ---- conv1 (batch-stacked): tmp1 [128=(bg,cf), (l1, b8)] ----
              tmp1 = work.tile([128, NB1], BF16)
              for ci in (range(2) if mode != 'empty' else []):
                n = NB1 // 2  # 496
                p1 = psum.tile([128, n], F32, tag="pchunk")
                nc.tensor.matmul(p1[:], cs_t[0:80, 0:128],
                                 a1_t[:, ci * n:(ci + 1) * n],
                                 start=True, stop=True)
                dst = tmp1[:, ci * n:(ci + 1) * n]
                if ci % 2 == 0:
                    nc.scalar.activation(dst, p1[:],
                                         mybir.ActivationFunctionType.Relu,
                                         bias=cs_t[:, 128:129])
                else:
                    nc.vector.tensor_scalar(dst, p1[:], cs_t[:, 128:129], 0.0,
                                            mybir.AluOpType.add,
                                            mybir.AluOpType.max)

              # ---- conv2 im2col: i2[k*16+c, (bg, l2, b8)] (5 fat DMAs) ----
              # src tmp1[bg*16+c, (l2+k)*8 + b8]: full-width partitions,
              # 3.75KB contiguous runs.
              i2 = work.tile([80, NB2], BF16)
              i2_3d = i2[:].rearrange("p (bg n) -> p bg n", bg=BG)
              if mode != 'empty':
                  for k in range(K):
                      for bg in range(BG):
                          nc.sync.dma_start(
                              i2_3d[k * C1:(k + 1) * C1, bg],
                              tmp1[bg * C1:(bg + 1) * C1,
                                   k * B8:k * B8 + L2 * B8])

              # ---- conv2 + relu -> x2r [128, (bg, l2, b8)] bf16 ----
              x2r = work.tile([C2, NB2], BF16)
              for ci in (range(NB2 // 512) if mode != 'empty' else []):
                p2 = psum.tile([128, 512], F32, tag="pchunk")
                nc.tensor.matmul(p2[:], sb_t[0:80, 0:128],
                                 i2[:, ci * 512:(ci + 1) * 512],
                                 start=True, stop=True)
                dst = x2r[:, ci * 512:(ci + 1) * 512]
                if ci % 2 == 0:
                    nc.scalar.activation(dst, p2[:],
                                         mybir.ActivationFunctionType.Relu,
                                         bias=cs_t[:, 129:130])
                else:
                    nc.vector.tensor_scalar(dst, p2[:], cs_t[:, 129:130], 0.0,
                                            mybir.AluOpType.add,
                                            mybir.AluOpType.max)

              # ---- conv3 (8 groups) + dense (1 group behind) ----
              pds = []
              for t in range(4):
                  pd_t = psumd.tile([128, 4 * B], F32, tag=f"pd{t}", name=f"pd{t}")
                  pds.append(pd_t)
              NQ = L3 // 4                 # 29 quads per group
              per_tile = G3 * NQ // 4      # 58 MMs per col-tile
              tile_seen = [0, 0, 0, 0]

              def emit_dense(zf, g):
                  zb = zf[0:128, 0:1]
                  zp, zf0 = zb.ap[0], zb.offset
                  for lq in range(NQ):
                      tj = (g * NQ + lq) % 4
                      wcol = 232 * (g % 2) + 8 * lq
                      # moving [128, (bg 8, q 4, b8 8)]: strided quad gather
                      mv = bass.AP(zb.tensor, zf0 + 4 * lq * B8,
                                   [zp, [L3 * B8, BG], [B8, 4], [1, B8]])
                      nc.tensor.matmul(pds[tj][32 * tj:32 * tj + 8, :],
                                       wt_ts[g // 2][:, wcol:wcol + 8],
                                       mv,
                                       start=(tile_seen[tj] == 0),
                                       stop=(tile_seen[tj] == per_tile - 1),
                                       tile_position=(0, 32 * tj))
                      tile_seen[tj] += 1

              chunks = [(i * 512, 512) for i in range(14)] + [(14 * 512, 256)]
              pend = []
              for g in (range(G3) if mode not in ('conv2stop', 'empty') else []):
                  i3 = i3pool.tile([80, NB3], BF16, tag="i3")
                  i3_3d = i3[:].rearrange("p (bg n) -> p bg n", bg=BG)
                  if mode != 'noim2col':
                      for k in range(K):
                          xb = x2r[g * C1:(g + 1) * C1, 0:L3 * B8]
                          src = bass.AP(xb.tensor, xb.offset + k * B8,
                                        [xb.ap[0], [L2 * B8, BG], [1, L3 * B8]])
                          nc.sync.dma_start(i3_3d[k * C1:(k + 1) * C1], src)
                  zf = zpool.tile([128, NB3], BF16, tag="zf")
                  for ci, (off, n) in enumerate(chunks):
                      p3 = psum.tile([128, n], F32, tag="pchunk")
                      nc.tensor.matmul(p3[:], sb_t[:, 640 + g * 128:640 + (g + 1) * 128],
                                       i3[:, off:off + n], start=True, stop=True)
                      dst = zf[:, off:off + n]
                      if ci % 2 == 0:
                          nc.scalar.activation(dst, p3[:],
                                               mybir.ActivationFunctionType.Relu,
                                               bias=cs_t[:, 130 + g:131 + g])
                      else:
                          nc.vector.tensor_scalar(dst, p3[:], cs_t[:, 130 + g:131 + g],
                                                  0.0, mybir.AluOpType.add,
                                                  mybir.AluOpType.max)
                  if mode != 'nodense':
                      pend.append((zf, g))
                      if len(pend) > 1:
                          emit_dense(*pend.pop(0))
              for args_ in pend:
                  emit_dense(*args_)

              # ---- write partials ----
              if mode == 'full':
                  out_t = work.tile([128, 4 * B], F32)
                  for tj in range(4):
                      nc.vector.tensor_copy(out_t[32 * tj:32 * tj + 8, :],
                                            pds[tj][32 * tj:32 * tj + 8, :])
                  for tj in range(4):
                      nc.sync.dma_start(out[32 * tj:32 * tj + 8, :],
                                        out_t[32 * tj:32 * tj + 8, :])
              else:
                  out_t = work.tile([128, 4 * B], F32)
                  nc.gpsimd.memset(out_t[:], 0.0)
                  nc.sync.dma_start(out[:], out_t[:])

            if rep_cm is not None:
                rep_cm.__exit__(None, None, None)

    nc.compile()
    return nc


def _shard_inputs(state, k1, b1, k2, b2, k3, b3, W, bd):
    """Host-side: build per-core input maps (layout only, no math)."""
    state = np.asarray(state, dtype=np.float32)
    k1 = np.asarray(k1, np.float32); b1 = np.asarray(b1, np.float32)
    k2 = np.asarray(k2, np.float32); b2 = np.asarray(b2, np.float32)
    k3 = np.asarray(k3, np.float32); b3 = np.asarray(b3, np.float32)
    W = np.asarray(W, np.float32)
    W3 = W.reshape(L3, C3 * N_CORES, 2)

    in_maps = []
    for j in range(N_CORES):
        x0 = state[:, :, C0 * j:C0 * (j + 1)]  # [B, L, 2]

        # conv1 im2col [80=(bg,k,c), (l1, b8)]
        a1 = np.zeros((80, L1 * B8), np.float32)
        for bg in range(BG):
            for k in range(K):
                for c in range(C0):
                    a1[bg * 10 + k * C0 + c] = (
                        x0[bg * B8:(bg + 1) * B8, k:k + L1, c].T.reshape(-1))
        # conv1 stationary blockdiag [80, 128=(bg,c,f)]
        s1 = np.zeros((80, 128), np.float32)
        for bg in range(BG):
            for c in range(C0):
                for k in range(K):
                    for f in range(FILTERS):
                        s1[bg * 10 + k * C0 + c,
                           bg * C1 + c * FILTERS + f] = k1[k, 0, (C0 * j + c) * FILTERS + f]
        b1p = np.tile(b1[C1 * j:C1 * (j + 1)], BG).astype(np.float32)  # [128]

        # conv2 tap stationaries: sb[k*16+c? rows (k,c)... rows k*16+c
        s2 = np.zeros((80, 128), np.float32)
        for k in range(K):
            for c in range(C1):
                for f in range(FILTERS):
                    s2[k * C1 + c, c * FILTERS + f] = k2[k, 0, (C1 * j + c) * FILTERS + f]
        b2p = b2[C2 * j:C2 * (j + 1)].astype(np.float32)  # [128]

        # conv3 stationaries [80=(k,c), 8x128]
        s3 = np.zeros((80, G3 * 128), np.float32)
        for g in range(G3):
            for k in range(K):
                for c in range(C1):
                    for f in range(FILTERS):
                        s3[k * C1 + c, g * 128 + c * FILTERS + f] = (
                            k3[k, 0, (C2 * j + C1 * g + c) * FILTERS + f])
        b3p = np.empty((128, G3), np.float32)
        for g in range(G3):
            b3p[:, g] = b3[(C2 * j + C1 * g) * FILTERS:
                           (C2 * j + C1 * g) * FILTERS + 128]

        cs = np.zeros((128, 138), np.float32)
        cs[0:80, 0:128] = s1
        cs[:, 128] = b1p
        cs[:, 129] = b2p
        cs[:, 130:138] = b3p

        sbm = np.zeros((80, 640 + G3 * 128), np.float32)
        # s2k slabs [16, 128] at cols k*128 (partitions 0:16? no --
        # conv2 stationary is [80,128] = all taps; keep as one [80,128])
        sbm[0:80, 0:128] = s2
        sbm[0:80, 640:640 + G3 * 128] = s3

        # dense weights [128p, (g, lq, q, a)] bf16
        W3s = W3[:, C3 * j:C3 * (j + 1), :].reshape(L3 // 4, 4, G3, 128, 2)
        wt = np.ascontiguousarray(W3s.transpose(3, 2, 0, 1, 4)).reshape(
            128, G3 * L3 * 2).astype(ml_dtypes.bfloat16)

        in_maps.append({"a1": a1, "cs": cs, "wt": wt,
                        "sb": sbm.astype(ml_dtypes.bfloat16)})
    return in_maps


def kernel(state, k1, b1, k2, b2, k3, b3, W, bd, **run_kwargs):
    if "nc" not in _CACHE:
        _CACHE["nc"] = _build_nc()
    nc = _CACHE["nc"]
    in_maps = _shard_inputs(state, k1, b1, k2, b2, k3, b3, W, bd)
    res = run_bass_kernel_spmd(nc, in_maps, list(range(N_CORES)), **run_kwargs)
    # device out [128, 256]: cols are (bg 8, q 4, b8 8);
    # partial[a, bg*8+b8] = sum_{tj,q} out[32*tj + 2*q + a, bg*32 + q*8 + b8]
    total = np.zeros((2, B), np.float32)
    for c in range(N_CORES):
        o = np.asarray(res.results[c]["out"]).reshape(128, BG, 4, B8)
        for tj in range(4):
            for q in range(4):
                total += o[32 * tj + 2 * q:32 * tj + 2 * q + 2, :, q, :].reshape(2, B)
    out = np.tanh(total.T + np.asarray(bd, np.float32)).astype(np.float32)
    if run_kwargs.get("trace"):
        _CACHE["last_result"] = res
    return out

